# revision 14
# baseline (speedup 1.0000x reference)
"""Trainium2 Bass kernel for masked GNN message passing (AdjacencyControl).

Computes, for fixed shapes N=100000 nodes, E edges, D=128 features:
    h   = x @ W.T + b
    out[i] = sum over edges (i, j) of (node_rankings[j] <= 10000) * h[j]

Strategy (8 NeuronCores, SPMD, no collectives):
  host: integer-only preprocessing — drop edges whose source is masked
        out, compact masked source nodes into a dense table, sort kept
        edges by destination, shard edges by destination range
        (N/8 nodes per core), pad each 128-row destination block to a
        fixed number of 128-edge chunks.
  core: (A) h_masked = x_masked @ W.T + b via TensorE, streamed to a
        DRAM table; (B) dma_gather pulls the per-edge source rows of
        the table into SBUF; (C) scatter-add realised as one-hot
        matmuls accumulated in PSUM per 128-row output block.
"""

import math
import os
import sys

import ml_dtypes
import numpy as np

for _p in ("/opt/trn_rl_repo", "/root/.axon_site/_ro/trn_rl_repo"):
    if os.path.isdir(_p) and _p not in sys.path:
        sys.path.append(_p)

import concourse.bass as bass
import concourse.mybir as mybir
import concourse.tile as tile
from concourse import bacc
from concourse.bass import ts
from concourse.bass_utils import run_bass_kernel_spmd

P = 128          # partitions / tile edge
D = 128          # feature dim
M = 8            # cores
K_RANK = 10000   # ranking threshold from the reference model

_cache: dict = {}
TRACE = False      # set True to capture an NTFF profile (slower dispatch)
LAST = {}          # exec_time_ns / profile info from the last run

# tuning knobs (env-overridable for experiments)
BF16 = os.environ.get("KBF16", "1") == "1"
GC = int(os.environ.get("KGC", "8"))       # chunks per dma_gather (<=8)
SINGLE_PACKET = os.environ.get("KSP", "1") == "1"
BF16NP = ml_dtypes.bfloat16


def _preprocess(x, W, b, edge_index, node_rankings):
    N = x.shape[0]
    nsh = -(-N // M)                    # nodes per core shard
    nsh_pad = -(-nsh // P) * P
    nblocks = nsh_pad // P

    mask = node_rankings <= K_RANK
    row = edge_index[0].astype(np.int64)
    col = edge_index[1].astype(np.int64)
    keep = mask[col]
    row = row[keep]
    col = col[keep]

    masked_nodes = np.flatnonzero(mask)
    nm = len(masked_nodes)
    nm_pad = max(P, -(-nm // P) * P)
    assert nm_pad <= 32512, (
        f"{nm} masked nodes exceeds the int16 gather-index capacity; "
        "this build only supports <=32512 masked source nodes"
    )
    remap = np.zeros(N, np.int64)
    remap[masked_nodes] = np.arange(nm)
    srcc = remap[col]

    order = np.argsort(row, kind="stable")
    row = row[order]
    srcc = srcc[order]

    core_of = row // nsh
    dst_local = row - core_of * nsh
    blk = dst_local // P
    gb = core_of * nblocks + blk                       # global block id
    counts = np.bincount(gb, minlength=M * nblocks)
    kc = max(2, -(-int(counts.max()) // P)) if len(row) else 2
    cap = kc * P

    group_start = np.zeros(M * nblocks, np.int64)
    np.cumsum(counts[:-1], out=group_start[1:])
    rank = np.arange(len(row)) - group_start[gb]
    slot = gb * cap + rank

    src_pad = np.zeros(M * nblocks * cap, np.int16)
    dstr_pad = np.full(M * nblocks * cap, -1.0, np.float32)
    src_pad[slot] = srcc.astype(np.int16)
    dstr_pad[slot] = (dst_local - blk * P).astype(np.float32)

    npad = nblocks * cap                               # padded edges per core
    nchunks = npad // P                                # = nblocks * kc

    # dma_gather index layout: index i lives at [partition i%16, free i//16],
    # replicated to all 8 groups of 16 partitions.
    gidx = src_pad.reshape(M, npad // 16, 16).transpose(0, 2, 1)
    gidx = np.ascontiguousarray(np.tile(gidx, (1, 8, 1)))

    # per-chunk destination offsets, partition-major: [M, 128, nchunks]
    cmp_dt = BF16NP if BF16 else np.float32
    dstr = np.ascontiguousarray(
        dstr_pad.reshape(M, nchunks, P).transpose(0, 2, 1)).astype(cmp_dt)

    xmt = np.zeros((D, nm_pad), np.float32)
    xmt[:, :nm] = x[masked_nodes].T
    wt = np.ascontiguousarray(W.T.astype(np.float32))
    bias = np.tile(b.astype(np.float32)[None, :], (P, 1))
    iota = np.tile(np.arange(P, dtype=np.float32)[None, :],
                   (P, 1)).astype(cmp_dt)

    meta = dict(
        N=N, nsh=nsh, nsh_pad=nsh_pad, nblocks=nblocks,
        nm_pad=nm_pad, kc=kc, nchunks=nchunks, npad=npad,
    )
    per_core = [
        {
            "xmt": xmt, "wt": wt, "bias": bias, "iota": iota,
            "gidx": np.ascontiguousarray(gidx[i]),
            "dstr": dstr[i],
        }
        for i in range(M)
    ]
    return meta, per_core


def _build(meta, stage=3):
    nm_pad = meta["nm_pad"]
    nsh_pad = meta["nsh_pad"]
    nblocks = meta["nblocks"]
    kc = meta["kc"]
    nchunks = meta["nchunks"]
    npad = meta["npad"]
    nt_h = nm_pad // P

    # SWDGE descriptor-ring limit: at most 1024 gather indices per
    # dma_gather instruction (HW-verified; 1280+ wedges the device).
    gc = min(GC, 8)                                    # chunks per gather
    ngather = -(-nchunks // gc)
    OB = 16                                            # out blocks per DMA

    f32 = mybir.dt.float32
    cdt = mybir.dt.bfloat16 if BF16 else f32    # compute/table dtype
    nc = bacc.Bacc("TRN2", target_bir_lowering=False, debug=False,
                   num_devices=M, num_swdge_queues=4)

    xmt_d = nc.declare_dram_parameter("xmt", [D, nm_pad], f32, isOutput=False)
    wt_d = nc.declare_dram_parameter("wt", [D, D], f32, isOutput=False)
    bias_d = nc.declare_dram_parameter("bias", [P, D], f32, isOutput=False)
    iota_d = nc.declare_dram_parameter("iota", [P, P], cdt, isOutput=False)
    gidx_d = nc.declare_dram_parameter(
        "gidx", [P, npad // 16], mybir.dt.int16, isOutput=False)
    dstr_d = nc.declare_dram_parameter(
        "dstr", [P, nchunks], cdt, isOutput=False)
    out_d = nc.declare_dram_parameter(
        "out", [nsh_pad, D], f32, isOutput=True)
    hm_d = nc.dram_tensor("hm", [nm_pad, D], cdt)

    HB = 4                                             # h tiles per store DMA

    with tile.TileContext(nc) as tc:
        with (
            tc.tile_pool(name="consts", bufs=1) as cpool,
            tc.tile_pool(name="xmt", bufs=1) as xpool,
            tc.tile_pool(name="hstage", bufs=3) as hpool,
            tc.tile_pool(name="msg", bufs=4) as mpool,
            tc.tile_pool(name="ptile", bufs=6) as ppool,
            tc.tile_pool(name="ostage", bufs=2) as opool,
            tc.tile_pool(name="psum_h", bufs=4, space="PSUM") as psh,
            tc.tile_pool(name="psum_o", bufs=4, space="PSUM") as pso,
        ):
            wt_t = cpool.tile([D, D], f32)
            nc.sync.dma_start(out=wt_t[:], in_=wt_d.ap())
            bias_t = cpool.tile([P, D], f32)
            nc.sync.dma_start(out=bias_t[:], in_=bias_d.ap())
            iota_t = cpool.tile([P, P], cdt)
            nc.sync.dma_start(out=iota_t[:], in_=iota_d.ap())
            gidx_t = cpool.tile([P, npad // 16], mybir.dt.int16)
            nc.sync.dma_start(out=gidx_t[:], in_=gidx_d.ap())
            dstr_t = cpool.tile([P, nchunks], cdt)
            nc.sync.dma_start(out=dstr_t[:], in_=dstr_d.ap())

            xmt_t = xpool.tile([D, nm_pad], f32)
            nslice = 4
            step = -(-nt_h // nslice) * P
            for s in range(0, nm_pad, step):
                e = min(s + step, nm_pad)
                nc.sync.dma_start(out=xmt_t[:, s:e], in_=xmt_d.ap()[:, s:e])

            # Phase A: h = x_masked @ W.T + b -> DRAM table, node-major
            hm_r = hm_d.ap().rearrange("(t p) d -> p t d", p=P)
            for t0 in range(0, nt_h, HB):
                t1 = min(t0 + HB, nt_h)
                hs = hpool.tile([P, HB, D], cdt, tag="hs")
                for t in range(t0, t1):
                    ph = psh.tile([P, D], f32)
                    nc.tensor.matmul(out=ph[:], lhsT=xmt_t[:, ts(t, P)],
                                     rhs=wt_t[:], start=True, stop=True)
                    nc.vector.tensor_add(out=hs[:, t - t0, :], in0=ph[:],
                                         in1=bias_t[:])
                nc.sync.dma_start(out=hm_r[:, t0:t1, :],
                                  in_=hs[:, : t1 - t0, :])

            # Phases B+C: gather message rows, one-hot matmul scatter-add
            out_r = out_d.ap().rearrange("(t p) d -> p t d", p=P)
            mbs = {}           # gather group -> (msg tile, chunk offset)
            ost = None
            o0 = 0             # first block staged in ost
            po = None
            for blk in range(nblocks):
                for j in range(kc):
                    c = blk * kc + j
                    gi = c // gc
                    if gi not in mbs:
                        c0 = gi * gc
                        nch = min(gc, nchunks - c0)
                        mb = mpool.tile([P, gc, D], cdt, tag="mb")
                        if stage >= 2:
                            nc.gpsimd.dma_gather(
                                out_ap=mb[:, :nch, :],
                                in_ap=hm_d.ap(),
                                idxs_ap=gidx_t[:, c0 * 8:(c0 + nch) * 8],
                                num_idxs=nch * P,
                                num_idxs_reg=nch * P,
                                elem_size=D,
                                queue_num=gi % 4,
                                single_packet=SINGLE_PACKET,
                            )
                        else:
                            nc.vector.memset(mb[:], 0.0)
                        mbs = {gi: (mb, c0)}
                    mb, c0 = mbs[gi]
                    if stage < 3:
                        continue
                    if j == 0:
                        po = pso.tile([P, D], f32)
                    pt = ppool.tile([P, P], cdt, tag="pt")
                    nc.vector.tensor_tensor(
                        out=pt[:],
                        in0=dstr_t[:, c:c + 1].to_broadcast([P, P]),
                        in1=iota_t[:],
                        op=mybir.AluOpType.is_equal,
                    )
                    nc.tensor.matmul(out=po[:], lhsT=pt[:],
                                     rhs=mb[:, c - c0, :],
                                     start=(j == 0), stop=(j == kc - 1))
                if ost is None:
                    ost = opool.tile([P, OB, D], f32, tag="ost")
                    o0 = blk
                if stage >= 3:
                    nc.vector.tensor_copy(out=ost[:, blk - o0, :], in_=po[:])
                else:
                    nc.vector.tensor_copy(out=ost[:, blk - o0, :],
                                          in_=mbs[next(iter(mbs))][0][:, 0, :])
                if blk - o0 == OB - 1 or blk == nblocks - 1:
                    nc.sync.dma_start(
                        out=out_r[:, o0:blk + 1, :],
                        in_=ost[:, :blk + 1 - o0, :])
                    ost = None

    nc.compile()
    return nc


def kernel(x, W, b, edge_index, node_rankings):
    x = np.asarray(x, dtype=np.float32)
    W = np.asarray(W, dtype=np.float32)
    b = np.asarray(b, dtype=np.float32)
    edge_index = np.asarray(edge_index)
    node_rankings = np.asarray(node_rankings)

    meta, per_core = _preprocess(x, W, b, edge_index, node_rankings)
    key = (meta["nm_pad"], meta["kc"], meta["nchunks"], meta["nsh_pad"])
    if key not in _cache:
        _cache[key] = _build(meta)
    nc = _cache[key]

    res = run_bass_kernel_spmd(nc, per_core, core_ids=list(range(M)),
                               trace=TRACE)
    LAST["exec_time_ns"] = res.exec_time_ns
    LAST["results"] = res
    outs = [res.results[i]["out"][: meta["nsh"]] for i in range(M)]
    full = np.concatenate(outs, axis=0)[: meta["N"]]
    return full.astype(np.float32)


# revision 16
# speedup vs baseline: 1.3217x; 1.3217x over previous
"""Trainium2 Bass kernel for masked GNN message passing (AdjacencyControl).

Computes, for fixed shapes N=100000 nodes, E edges, D=128 features:
    h   = x @ W.T + b
    out[i] = sum over edges (i, j) of (node_rankings[j] <= 10000) * h[j]

Strategy (8 NeuronCores, SPMD, no collectives):
  host: integer-only preprocessing — drop edges whose source is masked
        out, compact masked source nodes into a dense table, sort kept
        edges by destination, shard edges by destination range
        (N/8 nodes per core), pad each 128-row destination block to a
        fixed number of 128-edge chunks.
  core: (A) h_masked = x_masked @ W.T + b via TensorE, streamed to a
        DRAM table; (B) dma_gather pulls the per-edge source rows of
        the table into SBUF; (C) scatter-add realised as one-hot
        matmuls accumulated in PSUM per 128-row output block.
"""

import math
import os
import sys

import ml_dtypes
import numpy as np

for _p in ("/opt/trn_rl_repo", "/root/.axon_site/_ro/trn_rl_repo"):
    if os.path.isdir(_p) and _p not in sys.path:
        sys.path.append(_p)

import concourse.bass as bass
import concourse.mybir as mybir
import concourse.tile as tile
from concourse import bacc
from concourse.bass import ts
from concourse.bass_utils import run_bass_kernel_spmd

P = 128          # partitions / tile edge
D = 128          # feature dim
M = 8            # cores
K_RANK = 10000   # ranking threshold from the reference model

_cache: dict = {}
TRACE = False      # set True to capture an NTFF profile (slower dispatch)
LAST = {}          # exec_time_ns / profile info from the last run

# tuning knobs (env-overridable for experiments)
# table mode: "f32" = fp32 rows, "bf16" = bf16 256B rows, "dup" = bf16
# rows duplicated to 512B (keeps the 512B descriptor efficiency)
TBL = os.environ.get("KTBL", "dup")
BF16 = TBL != "f32"
GC = int(os.environ.get("KGC", "8"))       # chunks per dma_gather (<=8)
SINGLE_PACKET = os.environ.get("KSP", "1") == "1"
BF16NP = ml_dtypes.bfloat16


def _preprocess(x, W, b, edge_index, node_rankings):
    N = x.shape[0]
    nsh = -(-N // M)                    # nodes per core shard
    nsh_pad = -(-nsh // P) * P
    nblocks = nsh_pad // P

    mask = node_rankings <= K_RANK
    row = edge_index[0].astype(np.int64)
    col = edge_index[1].astype(np.int64)
    keep = mask[col]
    row = row[keep]
    col = col[keep]

    masked_nodes = np.flatnonzero(mask)
    nm = len(masked_nodes)
    nm_pad = max(P, -(-nm // P) * P)
    assert nm_pad <= 32512, (
        f"{nm} masked nodes exceeds the int16 gather-index capacity; "
        "this build only supports <=32512 masked source nodes"
    )
    remap = np.zeros(N, np.int64)
    remap[masked_nodes] = np.arange(nm)
    srcc = remap[col]

    order = np.argsort(row, kind="stable")
    row = row[order]
    srcc = srcc[order]

    core_of = row // nsh
    dst_local = row - core_of * nsh
    blk = dst_local // P
    gb = core_of * nblocks + blk                       # global block id
    counts = np.bincount(gb, minlength=M * nblocks)
    kc = max(2, -(-int(counts.max()) // P)) if len(row) else 2
    cap = kc * P

    group_start = np.zeros(M * nblocks, np.int64)
    np.cumsum(counts[:-1], out=group_start[1:])
    rank = np.arange(len(row)) - group_start[gb]
    slot = gb * cap + rank

    src_pad = np.zeros(M * nblocks * cap, np.int16)
    dstr_pad = np.full(M * nblocks * cap, -1.0, np.float32)
    src_pad[slot] = srcc.astype(np.int16)
    dstr_pad[slot] = (dst_local - blk * P).astype(np.float32)

    npad = nblocks * cap                               # padded edges per core
    nchunks = npad // P                                # = nblocks * kc

    # dma_gather index layout: index i lives at [partition i%16, free i//16],
    # replicated to all 8 groups of 16 partitions.
    gidx = src_pad.reshape(M, npad // 16, 16).transpose(0, 2, 1)
    gidx = np.ascontiguousarray(np.tile(gidx, (1, 8, 1)))

    # per-chunk destination offsets, partition-major: [M, 128, nchunks]
    cmp_dt = BF16NP if BF16 else np.float32
    dstr = np.ascontiguousarray(
        dstr_pad.reshape(M, nchunks, P).transpose(0, 2, 1)).astype(cmp_dt)

    xmt = np.zeros((D, nm_pad), np.float32)
    xmt[:, :nm] = x[masked_nodes].T
    wt = np.ascontiguousarray(W.T.astype(np.float32))
    bias = np.tile(b.astype(np.float32)[None, :], (P, 1))
    iota = np.tile(np.arange(P, dtype=np.float32)[None, :],
                   (P, 1)).astype(cmp_dt)

    meta = dict(
        N=N, nsh=nsh, nsh_pad=nsh_pad, nblocks=nblocks,
        nm_pad=nm_pad, kc=kc, nchunks=nchunks, npad=npad,
    )
    per_core = [
        {
            "xmt": xmt, "wt": wt, "bias": bias, "iota": iota,
            "gidx": np.ascontiguousarray(gidx[i]),
            "dstr": dstr[i],
        }
        for i in range(M)
    ]
    return meta, per_core


def _build(meta, stage=3):
    nm_pad = meta["nm_pad"]
    nsh_pad = meta["nsh_pad"]
    nblocks = meta["nblocks"]
    kc = meta["kc"]
    nchunks = meta["nchunks"]
    npad = meta["npad"]
    nt_h = nm_pad // P

    # SWDGE descriptor-ring limit: at most 1024 gather indices per
    # dma_gather instruction (HW-verified; 1280+ wedges the device).
    gc = min(GC, 8)                                    # chunks per gather
    ngather = -(-nchunks // gc)
    OB = 16                                            # out blocks per DMA

    f32 = mybir.dt.float32
    cdt = mybir.dt.bfloat16 if BF16 else f32    # compute/table dtype
    nc = bacc.Bacc("TRN2", target_bir_lowering=False, debug=False,
                   num_devices=M, num_swdge_queues=4)

    xmt_d = nc.declare_dram_parameter("xmt", [D, nm_pad], f32, isOutput=False)
    wt_d = nc.declare_dram_parameter("wt", [D, D], f32, isOutput=False)
    bias_d = nc.declare_dram_parameter("bias", [P, D], f32, isOutput=False)
    iota_d = nc.declare_dram_parameter("iota", [P, P], cdt, isOutput=False)
    gidx_d = nc.declare_dram_parameter(
        "gidx", [P, npad // 16], mybir.dt.int16, isOutput=False)
    dstr_d = nc.declare_dram_parameter(
        "dstr", [P, nchunks], cdt, isOutput=False)
    out_d = nc.declare_dram_parameter(
        "out", [nsh_pad, D], f32, isOutput=True)
    EW = 2 * D if TBL == "dup" else D              # table row width (elems)
    hm_d = nc.dram_tensor("hm", [nm_pad, EW], cdt)

    HB = 4                                             # h tiles per store DMA

    with tile.TileContext(nc) as tc:
        with (
            tc.tile_pool(name="consts", bufs=1) as cpool,
            tc.tile_pool(name="xmt", bufs=1) as xpool,
            tc.tile_pool(name="hstage", bufs=3) as hpool,
            tc.tile_pool(name="msg", bufs=4) as mpool,
            tc.tile_pool(name="ptile", bufs=6) as ppool,
            tc.tile_pool(name="ostage", bufs=2) as opool,
            tc.tile_pool(name="psum_h", bufs=4, space="PSUM") as psh,
            tc.tile_pool(name="psum_o", bufs=4, space="PSUM") as pso,
        ):
            wt_t = cpool.tile([D, D], f32)
            nc.sync.dma_start(out=wt_t[:], in_=wt_d.ap())
            bias_t = cpool.tile([P, D], f32)
            nc.sync.dma_start(out=bias_t[:], in_=bias_d.ap())
            iota_t = cpool.tile([P, P], cdt)
            nc.sync.dma_start(out=iota_t[:], in_=iota_d.ap())
            gidx_t = cpool.tile([P, npad // 16], mybir.dt.int16)
            nc.sync.dma_start(out=gidx_t[:], in_=gidx_d.ap())
            dstr_t = cpool.tile([P, nchunks], cdt)
            nc.sync.dma_start(out=dstr_t[:], in_=dstr_d.ap())

            xmt_t = xpool.tile([D, nm_pad], f32)
            nslice = 4
            step = -(-nt_h // nslice) * P
            for s in range(0, nm_pad, step):
                e = min(s + step, nm_pad)
                nc.sync.dma_start(out=xmt_t[:, s:e], in_=xmt_d.ap()[:, s:e])

            # Phase A: h = x_masked @ W.T + b -> DRAM table, node-major
            hm_r = hm_d.ap().rearrange("(t p) e -> p t e", p=P)
            for t0 in range(0, nt_h, HB):
                t1 = min(t0 + HB, nt_h)
                hs = hpool.tile([P, HB, D], cdt, tag="hs")
                for t in range(t0, t1):
                    ph = psh.tile([P, D], f32)
                    nc.tensor.matmul(out=ph[:], lhsT=xmt_t[:, ts(t, P)],
                                     rhs=wt_t[:], start=True, stop=True)
                    nc.vector.tensor_add(out=hs[:, t - t0, :], in0=ph[:],
                                         in1=bias_t[:])
                nc.sync.dma_start(out=hm_r[:, t0:t1, :D],
                                  in_=hs[:, : t1 - t0, :])
                if TBL == "dup":
                    nc.sync.dma_start(out=hm_r[:, t0:t1, D:],
                                      in_=hs[:, : t1 - t0, :])

            # Phases B+C: gather message rows, one-hot matmul scatter-add
            out_r = out_d.ap().rearrange("(t p) d -> p t d", p=P)
            mbs = {}           # gather group -> (msg tile, chunk offset)
            ost = None
            o0 = 0             # first block staged in ost
            po = None
            for blk in range(nblocks):
                for j in range(kc):
                    c = blk * kc + j
                    gi = c // gc
                    if gi not in mbs:
                        c0 = gi * gc
                        nch = min(gc, nchunks - c0)
                        mb = mpool.tile([P, gc, EW], cdt, tag="mb")
                        if stage >= 2:
                            nc.gpsimd.dma_gather(
                                out_ap=mb[:, :nch, :],
                                in_ap=hm_d.ap(),
                                idxs_ap=gidx_t[:, c0 * 8:(c0 + nch) * 8],
                                num_idxs=nch * P,
                                num_idxs_reg=nch * P,
                                elem_size=EW,
                                queue_num=gi % 4,
                                single_packet=SINGLE_PACKET,
                            )
                        else:
                            nc.vector.memset(mb[:], 0.0)
                        mbs = {gi: (mb, c0)}
                    mb, c0 = mbs[gi]
                    if stage < 3:
                        continue
                    if j == 0:
                        po = pso.tile([P, D], f32)
                    pt = ppool.tile([P, P], cdt, tag="pt")
                    nc.vector.tensor_tensor(
                        out=pt[:],
                        in0=dstr_t[:, c:c + 1].to_broadcast([P, P]),
                        in1=iota_t[:],
                        op=mybir.AluOpType.is_equal,
                    )
                    nc.tensor.matmul(out=po[:], lhsT=pt[:],
                                     rhs=mb[:, c - c0, :D],
                                     start=(j == 0), stop=(j == kc - 1))
                if ost is None:
                    ost = opool.tile([P, OB, D], f32, tag="ost")
                    o0 = blk
                if stage >= 3:
                    nc.vector.tensor_copy(out=ost[:, blk - o0, :], in_=po[:])
                else:
                    nc.vector.tensor_copy(out=ost[:, blk - o0, :],
                                          in_=mbs[next(iter(mbs))][0][:, 0, :])
                if blk - o0 == OB - 1 or blk == nblocks - 1:
                    nc.sync.dma_start(
                        out=out_r[:, o0:blk + 1, :],
                        in_=ost[:, :blk + 1 - o0, :])
                    ost = None

    nc.compile()
    return nc


def kernel(x, W, b, edge_index, node_rankings):
    x = np.asarray(x, dtype=np.float32)
    W = np.asarray(W, dtype=np.float32)
    b = np.asarray(b, dtype=np.float32)
    edge_index = np.asarray(edge_index)
    node_rankings = np.asarray(node_rankings)

    meta, per_core = _preprocess(x, W, b, edge_index, node_rankings)
    key = (meta["nm_pad"], meta["kc"], meta["nchunks"], meta["nsh_pad"])
    if key not in _cache:
        _cache[key] = _build(meta)
    nc = _cache[key]

    res = run_bass_kernel_spmd(nc, per_core, core_ids=list(range(M)),
                               trace=TRACE)
    LAST["exec_time_ns"] = res.exec_time_ns
    LAST["results"] = res
    outs = [res.results[i]["out"][: meta["nsh"]] for i in range(M)]
    full = np.concatenate(outs, axis=0)[: meta["N"]]
    return full.astype(np.float32)


# revision 23
# speedup vs baseline: 1.3763x; 1.0413x over previous
"""Trainium2 Bass kernel for masked GNN message passing (AdjacencyControl).

Computes, for fixed shapes N=100000 nodes, E edges, D=128 features:
    h   = x @ W.T + b
    out[i] = sum over edges (i, j) of (node_rankings[j] <= 10000) * h[j]

Strategy (8 NeuronCores, SPMD, no collectives):
  host: integer-only preprocessing — drop edges whose source is masked
        out, compact masked source nodes into a dense table, sort kept
        edges by destination, shard edges by destination range
        (N/8 nodes per core), pad each 128-row destination block to a
        fixed number of 128-edge chunks.
  core: (A) h_masked = x_masked @ W.T + b via TensorE, streamed to a
        DRAM table; (B) dma_gather pulls the per-edge source rows of
        the table into SBUF; (C) scatter-add realised as one-hot
        matmuls accumulated in PSUM per 128-row output block.
"""

import math
import os
import sys

import ml_dtypes
import numpy as np

for _p in ("/opt/trn_rl_repo", "/root/.axon_site/_ro/trn_rl_repo"):
    if os.path.isdir(_p) and _p not in sys.path:
        sys.path.append(_p)

import concourse.bass as bass
import concourse.mybir as mybir
import concourse.tile as tile
from concourse import bacc
from concourse.bass import ts
from concourse.bass_utils import run_bass_kernel_spmd

P = 128          # partitions / tile edge
D = 128          # feature dim
M = 8            # cores
K_RANK = 10000   # ranking threshold from the reference model

_cache: dict = {}
TRACE = False      # set True to capture an NTFF profile (slower dispatch)
LAST = {}          # exec_time_ns / profile info from the last run

# tuning knobs (env-overridable for experiments)
# table mode: "f32" = fp32 rows, "bf16" = bf16 256B rows, "dup" = bf16
# rows duplicated to 512B (keeps the 512B descriptor efficiency)
TBL = os.environ.get("KTBL", "dup")
BF16 = TBL != "f32"
GC = int(os.environ.get("KGC", "8"))       # chunks per dma_gather (<=8)
SINGLE_PACKET = os.environ.get("KSP", "1") == "1"
XCAST = os.environ.get("KXCAST", "1") == "1"   # xmt bf16 via SWDGE cast DMA
AB16 = os.environ.get("KAB16", "1") == "1"     # phase A matmul in bf16
DUMMY = os.environ.get("KDUMMY", "1") == "1"   # early ucode-lib preload
BF16NP = ml_dtypes.bfloat16


def _preprocess(x, W, b, edge_index, node_rankings):
    N = x.shape[0]
    nsh = -(-N // M)                    # nodes per core shard
    nsh_pad = -(-nsh // P) * P
    nblocks = nsh_pad // P

    mask = node_rankings <= K_RANK
    row = edge_index[0].astype(np.int64)
    col = edge_index[1].astype(np.int64)
    keep = mask[col]
    row = row[keep]
    col = col[keep]

    masked_nodes = np.flatnonzero(mask)
    nm = len(masked_nodes)
    nm_pad = max(P, -(-nm // P) * P)
    assert nm_pad <= 32512, (
        f"{nm} masked nodes exceeds the int16 gather-index capacity; "
        "this build only supports <=32512 masked source nodes"
    )
    remap = np.zeros(N, np.int64)
    remap[masked_nodes] = np.arange(nm)
    srcc = remap[col]

    order = np.argsort(row, kind="stable")
    row = row[order]
    srcc = srcc[order]

    core_of = row // nsh
    dst_local = row - core_of * nsh
    blk = dst_local // P
    gb = core_of * nblocks + blk                       # global block id
    counts = np.bincount(gb, minlength=M * nblocks)
    kc = max(2, -(-int(counts.max()) // P)) if len(row) else 2
    cap = kc * P

    group_start = np.zeros(M * nblocks, np.int64)
    np.cumsum(counts[:-1], out=group_start[1:])
    rank = np.arange(len(row)) - group_start[gb]
    slot = gb * cap + rank

    src_pad = np.zeros(M * nblocks * cap, np.int16)
    dstr_pad = np.full(M * nblocks * cap, -1.0, np.float32)
    src_pad[slot] = srcc.astype(np.int16)
    dstr_pad[slot] = (dst_local - blk * P).astype(np.float32)

    npad = nblocks * cap                               # padded edges per core
    nchunks = npad // P                                # = nblocks * kc

    # dma_gather index layout: index i lives at [partition i%16, free i//16],
    # replicated to all 8 groups of 16 partitions.
    gidx = src_pad.reshape(M, npad // 16, 16).transpose(0, 2, 1)
    gidx = np.ascontiguousarray(np.tile(gidx, (1, 8, 1)))

    # per-chunk destination offsets, partition-major: [M, 128, nchunks]
    cmp_dt = BF16NP if BF16 else np.float32
    dstr = np.ascontiguousarray(
        dstr_pad.reshape(M, nchunks, P).transpose(0, 2, 1)).astype(cmp_dt)

    xdt = np.float32 if (not BF16 or not AB16 or XCAST) else BF16NP
    xmt = np.zeros((D, nm_pad), xdt)
    xmt[:, :nm] = x[masked_nodes].T.astype(xdt)
    wt = np.ascontiguousarray(
        W.T.astype(BF16NP if (BF16 and AB16) else np.float32))
    bias = np.tile(b.astype(np.float32)[None, :], (P, 1))
    iota = np.tile(np.arange(P, dtype=np.float32)[None, :],
                   (P, 1)).astype(cmp_dt)

    meta = dict(
        N=N, nsh=nsh, nsh_pad=nsh_pad, nblocks=nblocks,
        nm_pad=nm_pad, kc=kc, nchunks=nchunks, npad=npad,
    )
    per_core = [
        {
            "xmt": xmt, "wt": wt, "bias": bias, "iota": iota,
            "gidx": np.ascontiguousarray(gidx[i]),
            "dstr": dstr[i],
        }
        for i in range(M)
    ]
    return meta, per_core


def _build(meta, stage=3):
    nm_pad = meta["nm_pad"]
    nsh_pad = meta["nsh_pad"]
    nblocks = meta["nblocks"]
    kc = meta["kc"]
    nchunks = meta["nchunks"]
    npad = meta["npad"]
    nt_h = nm_pad // P

    # SWDGE descriptor-ring limit: at most 1024 gather indices per
    # dma_gather instruction (HW-verified; 1280+ wedges the device).
    gc = min(GC, 8)                                    # chunks per gather
    ngather = -(-nchunks // gc)
    OB = 16                                            # out blocks per DMA

    f32 = mybir.dt.float32
    cdt = mybir.dt.bfloat16 if BF16 else f32    # compute/table dtype
    nc = bacc.Bacc("TRN2", target_bir_lowering=False, debug=False,
                   num_devices=M, num_swdge_queues=4)

    adt = cdt if AB16 else f32                  # phase A matmul dtype
    xmt_dt = f32 if (not BF16 or not AB16 or XCAST) else adt
    xmt_d = nc.declare_dram_parameter("xmt", [D, nm_pad], xmt_dt,
                                      isOutput=False)
    wt_d = nc.declare_dram_parameter("wt", [D, D], adt, isOutput=False)
    bias_d = nc.declare_dram_parameter("bias", [P, D], f32, isOutput=False)
    iota_d = nc.declare_dram_parameter("iota", [P, P], cdt, isOutput=False)
    gidx_d = nc.declare_dram_parameter(
        "gidx", [P, npad // 16], mybir.dt.int16, isOutput=False)
    dstr_d = nc.declare_dram_parameter(
        "dstr", [P, nchunks], cdt, isOutput=False)
    out_d = nc.declare_dram_parameter(
        "out", [nsh_pad, D], f32, isOutput=True)
    EW = 2 * D if TBL == "dup" else D              # table row width (elems)
    hm_d = nc.dram_tensor("hm", [nm_pad, EW], cdt)

    HB = 4                                             # h tiles per store DMA

    with tile.TileContext(nc) as tc:
        with (
            tc.tile_pool(name="consts", bufs=1) as cpool,
            tc.tile_pool(name="xmt", bufs=1) as xpool,
            tc.tile_pool(name="hstage", bufs=3) as hpool,
            tc.tile_pool(name="msg", bufs=int(os.environ.get("KMB", "8"))) as mpool,
            tc.tile_pool(name="ptile", bufs=6) as ppool,
            tc.tile_pool(name="ostage", bufs=2) as opool,
            tc.tile_pool(name="psum_h", bufs=4, space="PSUM") as psh,
            tc.tile_pool(name="psum_o", bufs=4, space="PSUM") as pso,
        ):
            wt_t = cpool.tile([D, D], adt)
            nc.sync.dma_start(out=wt_t[:], in_=wt_d.ap())
            bias_t = cpool.tile([P, D], f32)
            nc.sync.dma_start(out=bias_t[:], in_=bias_d.ap())
            iota_t = cpool.tile([P, P], cdt)
            nc.sync.dma_start(out=iota_t[:], in_=iota_d.ap())
            gidx_t = cpool.tile([P, npad // 16], mybir.dt.int16)
            nc.sync.dma_start(out=gidx_t[:], in_=gidx_d.ap())
            dstr_t = cpool.tile([P, nchunks], cdt)
            nc.sync.dma_start(out=dstr_t[:], in_=dstr_d.ap())

            xmt_t = xpool.tile([D, nm_pad], adt)
            nslice = int(os.environ.get("KNSL", "8"))
            step = -(-nt_h // nslice) * P
            dma_cast = (nc.gpsimd.dma_start
                        if (BF16 and AB16 and XCAST)
                        else nc.sync.dma_start)
            for s in range(0, nm_pad, step):
                e = min(s + step, nm_pad)
                dma_cast(out=xmt_t[:, s:e], in_=xmt_d.ap()[:, s:e])

            # tiny dummy gather: forces the GPSIMD ext-isa library load
            # early so it overlaps phase A instead of stalling the first
            # real gather
            if DUMMY:
                zidx_t = cpool.tile([P, 8], mybir.dt.int16)
                nc.vector.memset(zidx_t[:], 0)
                dummy = mpool.tile([P, 1, D], adt, tag="dummy")
                nc.gpsimd.dma_gather(
                    out_ap=dummy[:], in_ap=wt_d.ap(),
                    idxs_ap=zidx_t[:], num_idxs=P, num_idxs_reg=P,
                    elem_size=D, queue_num=3)

            # Phase A: h = x_masked @ W.T + b -> DRAM table, node-major
            hm_r = hm_d.ap().rearrange("(t p) e -> p t e", p=P)
            for t0 in range(0, nt_h, HB):
                t1 = min(t0 + HB, nt_h)
                hs = hpool.tile([P, HB, D], cdt, tag="hs")
                for t in range(t0, t1):
                    ph = psh.tile([P, D], f32)
                    nc.tensor.matmul(out=ph[:], lhsT=xmt_t[:, ts(t, P)],
                                     rhs=wt_t[:], start=True, stop=True)
                    nc.vector.tensor_add(out=hs[:, t - t0, :], in0=ph[:],
                                         in1=bias_t[:])
                nc.sync.dma_start(out=hm_r[:, t0:t1, :D],
                                  in_=hs[:, : t1 - t0, :])
                if TBL == "dup":
                    nc.sync.dma_start(out=hm_r[:, t0:t1, D:],
                                      in_=hs[:, : t1 - t0, :])

            # Phases B+C: gather message rows, one-hot matmul scatter-add
            out_r = out_d.ap().rearrange("(t p) d -> p t d", p=P)
            mbs = {}           # gather group -> (msg tile, chunk offset)
            ost = None
            o0 = 0             # first block staged in ost
            po = None
            for blk in range(nblocks):
                for j in range(kc):
                    c = blk * kc + j
                    gi = c // gc
                    if gi not in mbs:
                        c0 = gi * gc
                        nch = min(gc, nchunks - c0)
                        mb = mpool.tile([P, gc, EW], cdt, tag="mb")
                        if stage >= 2:
                            nc.gpsimd.dma_gather(
                                out_ap=mb[:, :nch, :],
                                in_ap=hm_d.ap(),
                                idxs_ap=gidx_t[:, c0 * 8:(c0 + nch) * 8],
                                num_idxs=nch * P,
                                num_idxs_reg=nch * P,
                                elem_size=EW,
                                queue_num=gi % 4,
                                single_packet=SINGLE_PACKET,
                            )
                        else:
                            nc.vector.memset(mb[:], 0.0)
                        mbs = {gi: (mb, c0)}
                    mb, c0 = mbs[gi]
                    if stage < 3:
                        continue
                    if j == 0:
                        po = pso.tile([P, D], f32)
                    pt = ppool.tile([P, P], cdt, tag="pt")
                    nc.vector.tensor_tensor(
                        out=pt[:],
                        in0=dstr_t[:, c:c + 1].to_broadcast([P, P]),
                        in1=iota_t[:],
                        op=mybir.AluOpType.is_equal,
                    )
                    nc.tensor.matmul(out=po[:], lhsT=pt[:],
                                     rhs=mb[:, c - c0, :D],
                                     start=(j == 0), stop=(j == kc - 1))
                if ost is None:
                    ost = opool.tile([P, OB, D], f32, tag="ost")
                    o0 = blk
                if stage >= 3:
                    nc.vector.tensor_copy(out=ost[:, blk - o0, :], in_=po[:])
                else:
                    nc.vector.tensor_copy(out=ost[:, blk - o0, :],
                                          in_=mbs[next(iter(mbs))][0][:, 0, :])
                if blk - o0 == OB - 1 or blk == nblocks - 1:
                    nc.sync.dma_start(
                        out=out_r[:, o0:blk + 1, :],
                        in_=ost[:, :blk + 1 - o0, :])
                    ost = None

    nc.compile()
    return nc


def kernel(x, W, b, edge_index, node_rankings):
    x = np.asarray(x, dtype=np.float32)
    W = np.asarray(W, dtype=np.float32)
    b = np.asarray(b, dtype=np.float32)
    edge_index = np.asarray(edge_index)
    node_rankings = np.asarray(node_rankings)

    meta, per_core = _preprocess(x, W, b, edge_index, node_rankings)
    key = (meta["nm_pad"], meta["kc"], meta["nchunks"], meta["nsh_pad"])
    if key not in _cache:
        _cache[key] = _build(meta)
    nc = _cache[key]

    res = run_bass_kernel_spmd(nc, per_core, core_ids=list(range(M)),
                               trace=TRACE)
    LAST["exec_time_ns"] = res.exec_time_ns
    LAST["results"] = res
    outs = [res.results[i]["out"][: meta["nsh"]] for i in range(M)]
    full = np.concatenate(outs, axis=0)[: meta["N"]]
    return full.astype(np.float32)


# revision 25
# speedup vs baseline: 1.4554x; 1.0575x over previous
"""Trainium2 Bass kernel for masked GNN message passing (AdjacencyControl).

Computes, for fixed shapes N=100000 nodes, E edges, D=128 features:
    h   = x @ W.T + b
    out[i] = sum over edges (i, j) of (node_rankings[j] <= 10000) * h[j]

Strategy (8 NeuronCores, SPMD, no collectives):
  host: integer-only preprocessing — drop edges whose source is masked
        out, compact masked source nodes into a dense table, sort kept
        edges by destination, shard edges by destination range
        (N/8 nodes per core), pad each 128-row destination block to a
        fixed number of 128-edge chunks.
  core: (A) h_masked = x_masked @ W.T + b via TensorE, streamed to a
        DRAM table; (B) dma_gather pulls the per-edge source rows of
        the table into SBUF; (C) scatter-add realised as one-hot
        matmuls accumulated in PSUM per 128-row output block.
"""

import math
import os
import sys

import ml_dtypes
import numpy as np

for _p in ("/opt/trn_rl_repo", "/root/.axon_site/_ro/trn_rl_repo"):
    if os.path.isdir(_p) and _p not in sys.path:
        sys.path.append(_p)

import concourse.bass as bass
import concourse.mybir as mybir
import concourse.tile as tile
from concourse import bacc
from concourse.bass import ts
from concourse.bass_utils import run_bass_kernel_spmd

P = 128          # partitions / tile edge
D = 128          # feature dim
M = 8            # cores
K_RANK = 10000   # ranking threshold from the reference model

_cache: dict = {}
TRACE = False      # set True to capture an NTFF profile (slower dispatch)
LAST = {}          # exec_time_ns / profile info from the last run

# tuning knobs (env-overridable for experiments)
# table mode: "f32" = fp32 rows, "bf16" = bf16 256B rows, "dup" = bf16
# rows duplicated to 512B (keeps the 512B descriptor efficiency)
TBL = os.environ.get("KTBL", "dup")
BF16 = TBL != "f32"
GC = int(os.environ.get("KGC", "8"))       # chunks per dma_gather (<=8)
SINGLE_PACKET = os.environ.get("KSP", "1") == "1"
XCAST = os.environ.get("KXCAST", "1") == "1"   # xmt bf16 via SWDGE cast DMA
AB16 = os.environ.get("KAB16", "1") == "1"     # phase A matmul in bf16
WTDEV = os.environ.get("KWTDEV", "0") == "1"   # cast W on device
LSTAGE = os.environ.get("KLSTAGE", "0") == "1" # stage lhsT in small tiles
DUMMY = os.environ.get("KDUMMY", "1") == "1"   # early ucode-lib preload
BF16NP = ml_dtypes.bfloat16


def _preprocess(x, W, b, edge_index, node_rankings):
    N = x.shape[0]
    nsh = -(-N // M)                    # nodes per core shard
    nsh_pad = -(-nsh // P) * P
    nblocks = nsh_pad // P

    mask = node_rankings <= K_RANK
    row = edge_index[0].astype(np.int64)
    col = edge_index[1].astype(np.int64)
    keep = mask[col]
    row = row[keep]
    col = col[keep]

    masked_nodes = np.flatnonzero(mask)
    nm = len(masked_nodes)
    nm_pad = max(P, -(-nm // P) * P)
    assert nm_pad <= 32512, (
        f"{nm} masked nodes exceeds the int16 gather-index capacity; "
        "this build only supports <=32512 masked source nodes"
    )
    remap = np.zeros(N, np.int64)
    remap[masked_nodes] = np.arange(nm)
    srcc = remap[col]

    order = np.argsort(row, kind="stable")
    row = row[order]
    srcc = srcc[order]

    core_of = row // nsh
    dst_local = row - core_of * nsh
    blk = dst_local // P
    gb = core_of * nblocks + blk                       # global block id
    counts = np.bincount(gb, minlength=M * nblocks)
    kc = max(2, -(-int(counts.max()) // P)) if len(row) else 2
    cap = kc * P

    group_start = np.zeros(M * nblocks, np.int64)
    np.cumsum(counts[:-1], out=group_start[1:])
    rank = np.arange(len(row)) - group_start[gb]
    slot = gb * cap + rank

    src_pad = np.zeros(M * nblocks * cap, np.int16)
    dstr_pad = np.full(M * nblocks * cap, -1.0, np.float32)
    src_pad[slot] = srcc.astype(np.int16)
    dstr_pad[slot] = (dst_local - blk * P).astype(np.float32)

    npad = nblocks * cap                               # padded edges per core
    nchunks = npad // P                                # = nblocks * kc

    # dma_gather index layout: index i lives at [partition i%16, free i//16],
    # replicated to all 8 groups of 16 partitions.
    gidx = src_pad.reshape(M, npad // 16, 16).transpose(0, 2, 1)
    gidx = np.ascontiguousarray(np.tile(gidx, (1, 8, 1)))

    # per-chunk destination offsets, partition-major: [M, 128, nchunks]
    cmp_dt = BF16NP if BF16 else np.float32
    dstr = np.ascontiguousarray(
        dstr_pad.reshape(M, nchunks, P).transpose(0, 2, 1)).astype(cmp_dt)

    xdt = np.float32 if (not BF16 or not AB16 or XCAST) else BF16NP
    xmt = np.zeros((D, nm_pad), xdt)
    xmt[:, :nm] = x[masked_nodes].T.astype(xdt)
    wt = np.ascontiguousarray(
        W.T.astype(BF16NP if (BF16 and AB16 and not WTDEV)
                   else np.float32))
    bias = np.tile(b.astype(np.float32)[None, :], (P, 1))
    iota = np.tile(np.arange(P, dtype=np.float32)[None, :],
                   (P, 1)).astype(cmp_dt)

    meta = dict(
        N=N, nsh=nsh, nsh_pad=nsh_pad, nblocks=nblocks,
        nm_pad=nm_pad, kc=kc, nchunks=nchunks, npad=npad,
    )
    per_core = [
        {
            "xmt": xmt, "wt": wt, "bias": bias, "iota": iota,
            "gidx": np.ascontiguousarray(gidx[i]),
            "dstr": dstr[i],
        }
        for i in range(M)
    ]
    return meta, per_core


def _build(meta, stage=3):
    nm_pad = meta["nm_pad"]
    nsh_pad = meta["nsh_pad"]
    nblocks = meta["nblocks"]
    kc = meta["kc"]
    nchunks = meta["nchunks"]
    npad = meta["npad"]
    nt_h = nm_pad // P

    # SWDGE descriptor-ring limit: at most 1024 gather indices per
    # dma_gather instruction (HW-verified; 1280+ wedges the device).
    gc = min(GC, 8)                                    # chunks per gather
    ngather = -(-nchunks // gc)
    OB = 16                                            # out blocks per DMA

    f32 = mybir.dt.float32
    cdt = mybir.dt.bfloat16 if BF16 else f32    # compute/table dtype
    nc = bacc.Bacc("TRN2", target_bir_lowering=False, debug=False,
                   num_devices=M, num_swdge_queues=4)

    adt = cdt if AB16 else f32                  # phase A matmul dtype
    xmt_dt = f32 if (not BF16 or not AB16 or XCAST) else adt
    xmt_d = nc.declare_dram_parameter("xmt", [D, nm_pad], xmt_dt,
                                      isOutput=False)
    wt_pdt = f32 if WTDEV else adt
    wt_d = nc.declare_dram_parameter("wt", [D, D], wt_pdt, isOutput=False)
    bias_d = nc.declare_dram_parameter("bias", [P, D], f32, isOutput=False)
    iota_d = nc.declare_dram_parameter("iota", [P, P], cdt, isOutput=False)
    gidx_d = nc.declare_dram_parameter(
        "gidx", [P, npad // 16], mybir.dt.int16, isOutput=False)
    dstr_d = nc.declare_dram_parameter(
        "dstr", [P, nchunks], cdt, isOutput=False)
    out_d = nc.declare_dram_parameter(
        "out", [nsh_pad, D], f32, isOutput=True)
    EW = 2 * D if TBL == "dup" else D              # table row width (elems)
    hm_d = nc.dram_tensor("hm", [nm_pad, EW], cdt)

    HB = 4                                             # h tiles per store DMA

    with tile.TileContext(nc) as tc:
        with (
            tc.tile_pool(name="consts", bufs=1) as cpool,
            tc.tile_pool(name="xmt", bufs=1) as xpool,
            tc.tile_pool(name="hstage", bufs=3) as hpool,
            tc.tile_pool(name="msg", bufs=int(os.environ.get("KMB", "8"))) as mpool,
            tc.tile_pool(name="ptile", bufs=6) as ppool,
            tc.tile_pool(name="ostage", bufs=2) as opool,
            tc.tile_pool(name="psum_h", bufs=4, space="PSUM") as psh,
            tc.tile_pool(name="psum_o", bufs=4, space="PSUM") as pso,
        ):
            if WTDEV and adt != f32:
                wt_raw = cpool.tile([D, D], f32)
                nc.sync.dma_start(out=wt_raw[:], in_=wt_d.ap())
                wt_t = cpool.tile([D, D], adt)
                nc.vector.tensor_copy(out=wt_t[:], in_=wt_raw[:])
            else:
                wt_t = cpool.tile([D, D], adt)
                nc.sync.dma_start(out=wt_t[:], in_=wt_d.ap())
            bias_t = cpool.tile([P, D], f32)
            nc.sync.dma_start(out=bias_t[:], in_=bias_d.ap())
            iota_t = cpool.tile([P, P], cdt)
            nc.sync.dma_start(out=iota_t[:], in_=iota_d.ap())
            gidx_t = cpool.tile([P, npad // 16], mybir.dt.int16)
            nc.sync.dma_start(out=gidx_t[:], in_=gidx_d.ap())
            dstr_t = cpool.tile([P, nchunks], cdt)
            nc.sync.dma_start(out=dstr_t[:], in_=dstr_d.ap())

            xmt_t = xpool.tile([D, nm_pad], adt)
            nslice = int(os.environ.get("KNSL", "8"))
            step = -(-nt_h // nslice) * P
            dma_cast = (nc.gpsimd.dma_start
                        if (BF16 and AB16 and XCAST)
                        else nc.sync.dma_start)
            for s in range(0, nm_pad, step):
                e = min(s + step, nm_pad)
                dma_cast(out=xmt_t[:, s:e], in_=xmt_d.ap()[:, s:e])

            # tiny dummy gather: forces the GPSIMD ext-isa library load
            # early so it overlaps phase A instead of stalling the first
            # real gather
            if DUMMY:
                zidx_t = cpool.tile([P, 8], mybir.dt.int16)
                nc.vector.memset(zidx_t[:], 0)
                dummy = mpool.tile([P, 1, D], cdt, tag="dummy")
                nc.gpsimd.dma_gather(
                    out_ap=dummy[:], in_ap=iota_d.ap(),
                    idxs_ap=zidx_t[:], num_idxs=P, num_idxs_reg=P,
                    elem_size=D, queue_num=3)

            # Phase A: h = x_masked @ W.T + b -> DRAM table, node-major
            hm_r = hm_d.ap().rearrange("(t p) e -> p t e", p=P)
            for t0 in range(0, nt_h, HB):
                t1 = min(t0 + HB, nt_h)
                hs = hpool.tile([P, HB, D], cdt, tag="hs")
                for t in range(t0, t1):
                    ph = psh.tile([P, D], f32)
                    if LSTAGE:
                        ls = hpool.tile([P, P], adt, tag="ls")
                        nc.vector.tensor_copy(out=ls[:],
                                              in_=xmt_t[:, ts(t, P)])
                        lhs_ap = ls[:]
                    else:
                        lhs_ap = xmt_t[:, ts(t, P)]
                    nc.tensor.matmul(out=lhs_ap and ph[:], lhsT=lhs_ap,
                                     rhs=wt_t[:], start=True, stop=True)
                    nc.vector.tensor_add(out=hs[:, t - t0, :], in0=ph[:],
                                         in1=bias_t[:])
                nc.sync.dma_start(out=hm_r[:, t0:t1, :D],
                                  in_=hs[:, : t1 - t0, :])
                if TBL == "dup":
                    nc.sync.dma_start(out=hm_r[:, t0:t1, D:],
                                      in_=hs[:, : t1 - t0, :])

            # Phases B+C: gather message rows, one-hot matmul scatter-add
            out_r = out_d.ap().rearrange("(t p) d -> p t d", p=P)
            mbs = {}           # gather group -> (msg tile, chunk offset)
            ost = None
            o0 = 0             # first block staged in ost
            po = None
            for blk in range(nblocks):
                for j in range(kc):
                    c = blk * kc + j
                    gi = c // gc
                    if gi not in mbs:
                        c0 = gi * gc
                        nch = min(gc, nchunks - c0)
                        mb = mpool.tile([P, gc, EW], cdt, tag="mb")
                        if stage >= 2:
                            nc.gpsimd.dma_gather(
                                out_ap=mb[:, :nch, :],
                                in_ap=hm_d.ap(),
                                idxs_ap=gidx_t[:, c0 * 8:(c0 + nch) * 8],
                                num_idxs=nch * P,
                                num_idxs_reg=nch * P,
                                elem_size=EW,
                                queue_num=gi % 4,
                                single_packet=SINGLE_PACKET,
                            )
                        else:
                            nc.vector.memset(mb[:], 0.0)
                        mbs = {gi: (mb, c0)}
                    mb, c0 = mbs[gi]
                    if stage < 3:
                        continue
                    if j == 0:
                        po = pso.tile([P, D], f32)
                    pt = ppool.tile([P, P], cdt, tag="pt")
                    nc.vector.tensor_tensor(
                        out=pt[:],
                        in0=dstr_t[:, c:c + 1].to_broadcast([P, P]),
                        in1=iota_t[:],
                        op=mybir.AluOpType.is_equal,
                    )
                    nc.tensor.matmul(out=po[:], lhsT=pt[:],
                                     rhs=mb[:, c - c0, :D],
                                     start=(j == 0), stop=(j == kc - 1))
                if ost is None:
                    ost = opool.tile([P, OB, D], f32, tag="ost")
                    o0 = blk
                if stage >= 3:
                    nc.vector.tensor_copy(out=ost[:, blk - o0, :], in_=po[:])
                else:
                    nc.vector.tensor_copy(out=ost[:, blk - o0, :],
                                          in_=mbs[next(iter(mbs))][0][:, 0, :])
                if blk - o0 == OB - 1 or blk == nblocks - 1:
                    nc.sync.dma_start(
                        out=out_r[:, o0:blk + 1, :],
                        in_=ost[:, :blk + 1 - o0, :])
                    ost = None

    nc.compile()
    return nc


def kernel(x, W, b, edge_index, node_rankings):
    x = np.asarray(x, dtype=np.float32)
    W = np.asarray(W, dtype=np.float32)
    b = np.asarray(b, dtype=np.float32)
    edge_index = np.asarray(edge_index)
    node_rankings = np.asarray(node_rankings)

    meta, per_core = _preprocess(x, W, b, edge_index, node_rankings)
    key = (meta["nm_pad"], meta["kc"], meta["nchunks"], meta["nsh_pad"])
    if key not in _cache:
        _cache[key] = _build(meta)
    nc = _cache[key]

    res = run_bass_kernel_spmd(nc, per_core, core_ids=list(range(M)),
                               trace=TRACE)
    LAST["exec_time_ns"] = res.exec_time_ns
    LAST["results"] = res
    outs = [res.results[i]["out"][: meta["nsh"]] for i in range(M)]
    full = np.concatenate(outs, axis=0)[: meta["N"]]
    return full.astype(np.float32)
